# revision 41
# baseline (speedup 1.0000x reference)
"""Causal multi-head attention layer on 8 Trainium2 NeuronCores.

Sharding: tensor-parallel over heads (16 heads -> 2 per core).
Per core, for its 2 heads:
  qkv^T = W_slice^T @ x^T         (f32r matmuls, x pre-transposed on host)
  S^T[k,q] = K^T_chunk^T @ Q^T    (scores transposed; softmax denom via
                                   ones-column folded into V stationary)
  att^T = exp(S^T/8)  (bf16, causal-trimmed + triangular mask on diagonal)
  out^T[dv,q] = (V|1)^T-stationary @ att^T   -> row 64 = denominator
  attout^T = out^T[0:64] * bcast(1/denom)
  partial^T[e,tok] = W_out_slice chunks @ attout^T   -> DRAM
Host: sum partials over cores, transpose, + b_out.
"""
import os
import numpy as np
import ml_dtypes

import concourse.bacc as bacc
import concourse.bass as bass
import concourse.mybir as mybir
import concourse.tile as tile
from concourse import bass_utils

B, S, E, H = 4, 2048, 1024, 16
D = E // H            # 64
TOK = B * S           # 8192
KC = E // 128         # 8 emb chunks
TB = 512              # qkv token block
QB = 1024             # attention q block
NB = S // TB          # 4 token blocks per batch
NQB = S // QB         # 2 q blocks per batch

f32 = mybir.dt.float32
f32r = mybir.dt.float32r
bf16 = mybir.dt.bfloat16
FT = mybir.ActivationFunctionType


def splits(lo, hi, step=512):
    """Split [lo, hi) into pieces aligned to `step` boundaries."""
    out = []
    p = lo
    while p < hi:
        q = min((p // step + 1) * step, hi)
        out.append((p, q))
        p = q
    return out


def build(repeats: int = 1, dbg: bool = False):
    nc = bacc.Bacc("TRN2", target_bir_lowering=False, debug=False, num_devices=8)
    xT = nc.dram_tensor("xT", [E, TOK], f32r, kind="ExternalInput")
    wq = nc.dram_tensor("wq", [E, 128], f32r, kind="ExternalInput")
    wk = nc.dram_tensor("wk", [E, 128], f32r, kind="ExternalInput")
    wv = nc.dram_tensor("wv", [E, 128], f32r, kind="ExternalInput")
    wo = nc.dram_tensor("wo", [128, E], f32r, kind="ExternalInput")
    bq = nc.dram_tensor("bq", [128, 1], f32, kind="ExternalInput")
    bk = nc.dram_tensor("bk", [128, 1], f32, kind="ExternalInput")
    bv = nc.dram_tensor("bv", [128, 1], f32, kind="ExternalInput")
    tri = nc.dram_tensor("tri", [128, 128], bf16, kind="ExternalInput")
    idd = nc.dram_tensor("idd", [128, 128], bf16, kind="ExternalInput")
    outp = nc.dram_tensor("outp", [E, TOK], f32, kind="ExternalOutput")
    if dbg:
        d_q = nc.dram_tensor("d_q", [128, S], f32, kind="ExternalOutput")
        d_k = nc.dram_tensor("d_k", [128, S], f32, kind="ExternalOutput")
        d_v = nc.dram_tensor("d_v", [128, S], f32, kind="ExternalOutput")
        d_att = nc.dram_tensor("d_att", [128, 8 * QB], f32, kind="ExternalOutput")
        d_ao = nc.dram_tensor("d_ao", [128, S], f32, kind="ExternalOutput")
        d_den = nc.dram_tensor("d_den", [1, QB], f32, kind="ExternalOutput")
        d_vn = nc.dram_tensor("d_vn", [128, 130], f32, kind="ExternalOutput")

    with tile.TileContext(nc) as tc:
        with (
            tc.tile_pool(name="wp", bufs=1) as wp,
            tc.tile_pool(name="xp", bufs=2) as xp,
            tc.tile_pool(name="qk", bufs=2) as qk,
            tc.tile_pool(name="vn", bufs=1) as vnp,
            tc.tile_pool(name="at", bufs=1) as atp,
            tc.tile_pool(name="ao", bufs=2) as aop,
            tc.tile_pool(name="ms", bufs=1) as ms,
            tc.tile_pool(name="op", bufs=3) as op,
            tc.tile_pool(name="psA", bufs=2, space="PSUM") as psA,
            tc.tile_pool(name="psS", bufs=1, space="PSUM") as psS,
            tc.tile_pool(name="psO", bufs=1, space="PSUM") as psO,
        ):
            # --- constants / weights (loaded once) ---
            # first QKV token block's x slice loads FIRST so the PE can
            # start as soon as wq lands; remaining weights follow.
            xt00 = []
            for hf in range(2):
                x1 = xp.tile([128, KC * TB // 2], f32r, tag=f"xt{hf}",
                             name=f"xt_pre0_{hf}")
                nc.sync.dma_start(
                    x1[:].rearrange("p (c m) -> p c m", c=KC // 2),
                    xT.ap()[hf * (E // 2):(hf + 1) * (E // 2), 0:TB].rearrange(
                        "(c p) m -> p c m", p=128),
                )
                xt00.append(x1)
            wq_sb = wp.tile([128, E], f32r)
            wk_sb = wp.tile([128, E], f32r)
            wv_sb = wp.tile([128, E], f32r)
            wo_sb = wp.tile([128, E], f32r)
            bq_sb = wp.tile([128, 1], f32)
            bk_sb = wp.tile([128, 1], f32)
            bv_sb = wp.tile([128, 1], f32)
            for hf in range(2):
                nc.sync.dma_start(
                    wq_sb[:, hf * (E // 2):(hf + 1) * (E // 2)].rearrange(
                        "p (c m) -> p c m", c=KC // 2),
                    wq.ap()[hf * (E // 2):(hf + 1) * (E // 2), :].rearrange(
                        "(c p) m -> p c m", p=128),
                )
            nc.sync.dma_start(bq_sb[:], bq.ap())
            for wsb_, wdr_ in ((wk_sb, wk), (wv_sb, wv)):
                nc.sync.dma_start(
                    wsb_[:].rearrange("p (c m) -> p c m", c=KC),
                    wdr_.ap().rearrange("(c p) m -> p c m", p=128),
                )
            nc.sync.dma_start(wo_sb[:], wo.ap())
            nc.sync.dma_start(bk_sb[:], bk.ap())
            nc.sync.dma_start(bv_sb[:], bv.ap())
            tri_sb = wp.tile([128, 128], bf16)
            nc.sync.dma_start(tri_sb[:], tri.ap())
            id_sb = wp.tile([128, 128], bf16)
            nc.sync.dma_start(id_sb[:], idd.ap())
            # preload ACT exp table set during the prologue
            warm = wp.tile([1, 1], f32)
            nc.vector.memset(warm[:], 0.0)
            nc.scalar.activation(warm[:], warm[:], FT.Exp, scale=1.0)
            # persistent V-natural tiles; ones columns written once
            vns = []
            for i in range(S // 128):
                vn = vnp.tile([128, 130], bf16, tag=f"vn{i}", name=f"vn{i}")
                nc.vector.memset(vn[:, 64:65], 1.0)
                nc.vector.memset(vn[:, 129:130], 1.0)
                vns.append(vn)

            def alloc_qkv(b):
                return (
                    qk.tile([128, S], f32r, tag="qT", name=f"qT{b}"),
                    qk.tile([128, S], f32r, tag="kT", name=f"kT{b}"),
                    qk.tile([128, S], bf16, tag="vT", name=f"vT{b}"),
                )

            def qkv_dma(b, t, rep):
                tok0 = b * S + t * TB
                xth = []
                for hf in range(2):
                    x1 = xp.tile([128, KC * TB // 2], f32r, tag=f"xt{hf}",
                                 name=f"xt{rep}_{b}_{t}_{hf}")
                    nc.sync.dma_start(
                        x1[:].rearrange("p (c m) -> p c m", c=KC // 2),
                        xT.ap()[hf * (E // 2):(hf + 1) * (E // 2),
                                tok0:tok0 + TB].rearrange(
                            "(c p) m -> p c m", p=128),
                    )
                    xth.append(x1)
                return xth

            def qkv_group(b, t, tiles, xth, gi, rep):
                qT, kT, vT = tiles
                wsb, bsb, dst = (
                    (wq_sb, bq_sb, qT), (wk_sb, bk_sb, kT),
                    (wv_sb, bv_sb, vT),
                )[gi]
                ps = psA.tile([128, TB], f32, tag="mm512",
                              name=f"psqkv{rep}_{b}_{t}_{gi}")
                for kc in range(KC):
                    xsrc = xth[kc // (KC // 2)]
                    nc.tensor.matmul(
                        ps[:],
                        wsb[:, kc * 128:(kc + 1) * 128],
                        xsrc[:, (kc % (KC // 2)) * TB:
                             (kc % (KC // 2) + 1) * TB],
                        start=(kc == 0), stop=(kc == KC - 1),
                    )
                nc.vector.tensor_scalar_add(
                    dst[:, t * TB:(t + 1) * TB], ps[:], bsb[:]
                )

            def vnat(b, tiles, rep, lo=0, hi=S // 128):
                vT = tiles[2]
                for i in range(lo, hi):
                    vn = vns[i]
                    pst = psA.tile([128, 128], bf16, tag="mm512",
                                   name=f"pst{rep}_{b}_{i}")
                    nc.tensor.transpose(
                        pst[:], vT[:, i * 128:(i + 1) * 128], id_sb[:]
                    )
                    # one copy into both 64-col head groups (skips the ones
                    # columns at 64 / 129) via 3D APs
                    dst = vn[:, 0:64]
                    dst3 = bass.AP(dst.tensor, dst.offset,
                                   [dst.ap[0], [65, 2], [1, 64]])
                    src = pst[:, 0:64]
                    src3 = bass.AP(src.tensor, src.offset,
                                   [src.ap[0], [64, 2], [1, 64]])
                    nc.vector.tensor_copy(dst3, src3)

            def scores(b, qb, tiles, att, rep, fill=()):
                qT, kT, vT = tiles
                q0 = qb * QB
                nkc = (q0 + QB) // 128
                fill = list(fill)
                nf = len(fill)
                fired = 0
                pss = {}
                for kc in range(nkc):
                    kst = kc * 128
                    r0 = max(0, kst - q0)
                    for h in range(2):
                        ps_s = psS.tile([128, QB], f32, tag=f"s{h}",
                                        name=f"pss{rep}_{b}_{qb}_{kc}_{h}")
                        hs = slice(h * 64, (h + 1) * 64)
                        for (p0, p1) in splits(r0, QB):
                            nc.tensor.matmul(
                                ps_s[:, p0:p1],
                                kT[hs, kst:kst + 128],
                                qT[hs, q0 + p0:q0 + p1],
                                start=True, stop=True,
                                tile_position=(h * 64, 0),
                            )
                        pss[(kc, h)] = ps_s
                    for h in range(2):
                        ps_s = pss[(kc, h)]
                        nc.scalar.activation(
                            att[h][:, kc * QB + r0:(kc + 1) * QB],
                            ps_s[:, r0:QB],
                            FT.Exp, scale=0.125,
                        )
                        if kst >= q0:
                            blk = att[h][:, kc * QB + r0:kc * QB + r0 + 128]
                            nc.vector.tensor_tensor(
                                blk, blk, tri_sb[:],
                                op=mybir.AluOpType.mult,
                            )
                    # interleave PE fill work (next batch QKV groups) to
                    # cover the ACT exp-throughput deficit
                    want = (kc + 1) * nf // nkc
                    while fired < want:
                        fill[fired]()
                        fired += 1
                while fired < nf:
                    fill[fired]()
                    fired += 1

            def attv_qbb(b, qb, qbb, h, att, aos, rep):
                q0 = qb * QB
                qa0 = q0 + qbb * 512
                nkc_q = (qa0 + 512) // 128
                ps_o = psO.tile([65, 512], f32, tag=f"o{h}",
                                name=f"pso{rep}_{b}_{qb}_{qbb}_{h}")
                for kc in range(nkc_q):
                    kst = kc * 128
                    lo = max(qa0, kst) - qa0
                    vn = vns[kc]
                    nc.tensor.matmul(
                        ps_o[:, lo:512],
                        vn[:, h * 65:(h + 1) * 65],
                        att[h][:, kc * QB + qbb * 512 + lo:
                               kc * QB + (qbb + 1) * 512],
                        start=(kc == 0), stop=(kc == nkc_q - 1),
                    )
                rec = ms.tile([1, 512], f32, tag=f"rec{h}",
                              name=f"rec{rep}_{b}_{qb}_{qbb}_{h}")
                nc.vector.reciprocal(rec[:], ps_o[64:65, :])
                bc = ms.tile([64, 512], f32, tag=f"bc{h}",
                             name=f"bc{rep}_{b}_{qb}_{qbb}_{h}")
                nc.gpsimd.partition_broadcast(bc[:], rec[:])
                nc.vector.tensor_tensor(
                    aos[h * 64:(h + 1) * 64, qa0:qa0 + 512],
                    ps_o[0:64, :], bc[:],
                    op=mybir.AluOpType.mult,
                )

            def outproj_ec(b, half, ec, aos, rep, eng="alt"):
                t0b = b * S
                po = op.tile([128, S // 2], f32, tag="po",
                             name=f"po{rep}_{b}_{ec}_{half}")
                for tt in range(NB // 2):
                    t = half * (NB // 2) + tt
                    ps_p = psA.tile([128, TB], f32, tag="mm512",
                                    name=f"psp{rep}_{b}_{ec}_{t}")
                    nc.tensor.matmul(
                        ps_p[:],
                        wo_sb[:, ec * 128:(ec + 1) * 128],
                        aos[:, t * TB:(t + 1) * TB],
                        start=True, stop=True,
                    )
                    # copy engine: ScalarE only when not competing with
                    # the scores-loop exp FIFO
                    if eng == "alt" and (ec * 2 + tt) % 2 == 1:
                        nc.scalar.copy(
                            po[:, tt * TB:(tt + 1) * TB], ps_p[:]
                        )
                    else:
                        nc.vector.tensor_copy(
                            po[:, tt * TB:(tt + 1) * TB], ps_p[:]
                        )
                nc.sync.dma_start(
                    outp.ap()[ec * 128:(ec + 1) * 128,
                              t0b + half * (S // 2):
                              t0b + (half + 1) * (S // 2)],
                    po[:],
                )

            def outproj_half(b, half, aos, rep):
                for ec in range(KC):
                    outproj_ec(b, half, ec, aos, rep)

            for rep in range(repeats):
                # prologue: only batch-0 tokens [0,1024) serially; the rest
                # becomes fill work inside the first scores loop
                tiles = alloc_qkv(0)
                for t in (0, 1):
                    xth = xt00 if (t == 0 and rep == 0) else qkv_dma(0, t, rep)
                    for gi in range(3):
                        qkv_group(0, t, tiles, xth, gi, rep)
                vnat(0, tiles, rep, 0, 8)
                pro_fill = []
                for t in (2, 3):
                    xth = qkv_dma(0, t, rep)
                    for gi in range(3):
                        pro_fill.append(
                            (lambda t=t, xth=xth, gi=gi, tl=tiles:
                             qkv_group(0, t, tl, xth, gi, rep))
                        )
                pro_fill.append(
                    (lambda tl=tiles: vnat(0, tl, rep, 8, S // 128))
                )
                prev = None  # (b, aos) with half-1 outproj still pending
                for b in range(B):
                    nxt = b + 1 if b + 1 < B else None
                    tiles_next = alloc_qkv(nxt) if nxt is not None else None
                    aos = aop.tile([128, S], f32r, tag="ao", name=f"ao{rep}_{b}")
                    for qb in range(NQB):
                        att = [
                            atp.tile([128, 16 * QB], bf16, tag=f"att{h}",
                                     name=f"att{rep}_{b}_{qb}_{h}")
                            for h in range(2)
                        ]
                        fill = []
                        if b == 0 and qb == 0:
                            fill.extend(pro_fill)
                        if nxt is not None:
                            tls = [0] if qb == 0 else [1, 2, 3]
                            for t in tls:
                                xth = qkv_dma(nxt, t, rep)
                                for gi in range(3):
                                    fill.append(
                                        (lambda t=t, xth=xth, gi=gi:
                                         qkv_group(nxt, t, tiles_next,
                                                   xth, gi, rep))
                                    )
                        if qb == 0 and prev is not None:
                            pb, paos = prev
                            for ec in range(KC):
                                fill.append(
                                    (lambda ec=ec, pb=pb, paos=paos:
                                     outproj_ec(pb, 1, ec, paos, rep,
                                                eng="alt"))
                                )
                            prev = None
                        if qb == 1:
                            for ec in range(KC):
                                fill.append(
                                    (lambda ec=ec: outproj_ec(b, 0, ec,
                                                              aos, rep,
                                                              eng="alt"))
                                )
                        scores(b, qb, tiles, att, rep, fill)
                        for qbb in range(QB // 512):
                            for h in range(2):
                                attv_qbb(b, qb, qbb, h, att, aos, rep)
                    if nxt is not None:
                        vnat(nxt, tiles_next, rep)
                    prev = (b, aos)
                    tiles = tiles_next
                pb, paos = prev
                outproj_half(pb, 1, paos, rep)
    nc.compile()
    return nc


_CACHE = {}


def _get_nc(repeats=1):
    if repeats not in _CACHE:
        _CACHE[repeats] = build(repeats)
    return _CACHE[repeats]


def make_in_maps(x, W_qkv, b_qkv, W_out, b_out):
    x = np.asarray(x, dtype=np.float32)
    W_qkv = np.asarray(W_qkv, dtype=np.float32)
    b_qkv = np.asarray(b_qkv, dtype=np.float32)
    W_out = np.asarray(W_out, dtype=np.float32)
    xT = np.ascontiguousarray(x.reshape(TOK, E).T)
    trim = np.ascontiguousarray(
        np.triu(np.ones((128, 128), dtype=np.float32))
    ).astype(ml_dtypes.bfloat16)
    in_maps = []
    for c in range(8):
        cs = slice(c * 128, (c + 1) * 128)
        in_maps.append({
            "xT": xT,
            "wq": np.ascontiguousarray(W_qkv[:, c * 128:(c + 1) * 128]),
            "wk": np.ascontiguousarray(W_qkv[:, E + c * 128:E + (c + 1) * 128]),
            "wv": np.ascontiguousarray(
                W_qkv[:, 2 * E + c * 128:2 * E + (c + 1) * 128]),
            "wo": np.ascontiguousarray(W_out[cs, :]),
            "bq": np.ascontiguousarray(b_qkv[c * 128:(c + 1) * 128, None]),
            "bk": np.ascontiguousarray(b_qkv[E + c * 128:E + (c + 1) * 128, None]),
            "bv": np.ascontiguousarray(
                b_qkv[2 * E + c * 128:2 * E + (c + 1) * 128, None]),
            "tri": trim,
            "idd": np.eye(128, dtype=np.float32).astype(ml_dtypes.bfloat16),
        })
    return in_maps


def gather(results, b_out):
    total = np.zeros((E, TOK), dtype=np.float64)
    for c in range(8):
        total += results[c]["outp"].astype(np.float64)
    out = total.T.astype(np.float32) + np.asarray(b_out, dtype=np.float32)
    return np.ascontiguousarray(out.reshape(B, S, E)).astype(np.float32)


def kernel(x, W_qkv, b_qkv, W_out, b_out):
    nc = _get_nc(1)
    in_maps = make_in_maps(x, W_qkv, b_qkv, W_out, b_out)
    res = bass_utils.run_bass_kernel_spmd(nc, in_maps, core_ids=list(range(8)))
    return gather(res.results, b_out)


# revision 42
# speedup vs baseline: 1.2447x; 1.2447x over previous
"""Causal multi-head attention layer on 8 Trainium2 NeuronCores.

Sharding: tensor-parallel over heads (16 heads -> 2 per core).
Per core, for its 2 heads:
  qkv^T = W_slice^T @ x^T         (f32r matmuls, x pre-transposed on host)
  S^T[k,q] = K^T_chunk^T @ Q^T    (scores transposed; softmax denom via
                                   ones-column folded into V stationary)
  att^T = exp(S^T/8)  (bf16, causal-trimmed + triangular mask on diagonal)
  out^T[dv,q] = (V|1)^T-stationary @ att^T   -> row 64 = denominator
  attout^T = out^T[0:64] * bcast(1/denom)
  partial^T[e,tok] = W_out_slice chunks @ attout^T   -> DRAM
Host: sum partials over cores, transpose, + b_out.
"""
import os
import numpy as np
import ml_dtypes

import concourse.bacc as bacc
import concourse.bass as bass
import concourse.mybir as mybir
import concourse.tile as tile
from concourse import bass_utils

B, S, E, H = 4, 2048, 1024, 16
D = E // H            # 64
TOK = B * S           # 8192
KC = E // 128         # 8 emb chunks
TB = 512              # qkv token block
QB = 1024             # attention q block
NB = S // TB          # 4 token blocks per batch
NQB = S // QB         # 2 q blocks per batch

f32 = mybir.dt.float32
f32r = mybir.dt.float32r
bf16 = mybir.dt.bfloat16
FT = mybir.ActivationFunctionType


def splits(lo, hi, step=512):
    """Split [lo, hi) into pieces aligned to `step` boundaries."""
    out = []
    p = lo
    while p < hi:
        q = min((p // step + 1) * step, hi)
        out.append((p, q))
        p = q
    return out


def build(repeats: int = 1, dbg: bool = False):
    nc = bacc.Bacc("TRN2", target_bir_lowering=False, debug=False, num_devices=8)
    xT = nc.dram_tensor("xT", [E, TOK], f32r, kind="ExternalInput")
    wq = nc.dram_tensor("wq", [E, 128], f32r, kind="ExternalInput")
    wk = nc.dram_tensor("wk", [E, 128], f32r, kind="ExternalInput")
    wv = nc.dram_tensor("wv", [E, 128], f32r, kind="ExternalInput")
    wo = nc.dram_tensor("wo", [128, E], f32r, kind="ExternalInput")
    bq = nc.dram_tensor("bq", [128, 1], f32, kind="ExternalInput")
    bk = nc.dram_tensor("bk", [128, 1], f32, kind="ExternalInput")
    bv = nc.dram_tensor("bv", [128, 1], f32, kind="ExternalInput")
    tri = nc.dram_tensor("tri", [128, 128], bf16, kind="ExternalInput")
    idd = nc.dram_tensor("idd", [128, 128], bf16, kind="ExternalInput")
    outp = nc.dram_tensor("outp", [E, TOK], f32, kind="ExternalOutput")
    if dbg:
        d_q = nc.dram_tensor("d_q", [128, S], f32, kind="ExternalOutput")
        d_k = nc.dram_tensor("d_k", [128, S], f32, kind="ExternalOutput")
        d_v = nc.dram_tensor("d_v", [128, S], f32, kind="ExternalOutput")
        d_att = nc.dram_tensor("d_att", [128, 8 * QB], f32, kind="ExternalOutput")
        d_ao = nc.dram_tensor("d_ao", [128, S], f32, kind="ExternalOutput")
        d_den = nc.dram_tensor("d_den", [1, QB], f32, kind="ExternalOutput")
        d_vn = nc.dram_tensor("d_vn", [128, 130], f32, kind="ExternalOutput")

    with tile.TileContext(nc) as tc:
        with (
            tc.tile_pool(name="wp", bufs=1) as wp,
            tc.tile_pool(name="xp", bufs=2) as xp,
            tc.tile_pool(name="qk", bufs=2) as qk,
            tc.tile_pool(name="vn", bufs=1) as vnp,
            tc.tile_pool(name="at", bufs=1) as atp,
            tc.tile_pool(name="ao", bufs=2) as aop,
            tc.tile_pool(name="ms", bufs=1) as ms,
            tc.tile_pool(name="op", bufs=3) as op,
            tc.tile_pool(name="psA", bufs=2, space="PSUM") as psA,
            tc.tile_pool(name="psS", bufs=1, space="PSUM") as psS,
            tc.tile_pool(name="psO", bufs=1, space="PSUM") as psO,
        ):
            # --- constants / weights (loaded once) ---
            # first QKV token block's x slice loads FIRST so the PE can
            # start as soon as wq lands; remaining weights follow.
            xt00 = []
            for hf in range(2):
                x1 = xp.tile([128, KC * TB // 2], f32r, tag=f"xt{hf}",
                             name=f"xt_pre0_{hf}")
                nc.sync.dma_start(
                    x1[:].rearrange("p (c m) -> p c m", c=KC // 2),
                    xT.ap()[hf * (E // 2):(hf + 1) * (E // 2), 0:TB].rearrange(
                        "(c p) m -> p c m", p=128),
                )
                xt00.append(x1)
            wq_sb = wp.tile([128, E], f32r)
            wk_sb = wp.tile([128, E], f32r)
            wv_sb = wp.tile([128, E], f32r)
            wo_sb = wp.tile([128, E], f32r)
            bq_sb = wp.tile([128, 1], f32)
            bk_sb = wp.tile([128, 1], f32)
            bv_sb = wp.tile([128, 1], f32)
            for hf in range(2):
                nc.sync.dma_start(
                    wq_sb[:, hf * (E // 2):(hf + 1) * (E // 2)].rearrange(
                        "p (c m) -> p c m", c=KC // 2),
                    wq.ap()[hf * (E // 2):(hf + 1) * (E // 2), :].rearrange(
                        "(c p) m -> p c m", p=128),
                )
            nc.sync.dma_start(bq_sb[:], bq.ap())
            for wsb_, wdr_ in ((wk_sb, wk), (wv_sb, wv)):
                nc.sync.dma_start(
                    wsb_[:].rearrange("p (c m) -> p c m", c=KC),
                    wdr_.ap().rearrange("(c p) m -> p c m", p=128),
                )
            nc.sync.dma_start(wo_sb[:], wo.ap())
            nc.sync.dma_start(bk_sb[:], bk.ap())
            nc.sync.dma_start(bv_sb[:], bv.ap())
            tri_sb = wp.tile([128, 128], bf16)
            nc.sync.dma_start(tri_sb[:], tri.ap())
            id_sb = wp.tile([128, 128], bf16)
            nc.sync.dma_start(id_sb[:], idd.ap())
            # preload ACT exp table set during the prologue
            warm = wp.tile([1, 1], f32)
            nc.vector.memset(warm[:], 0.0)
            nc.scalar.activation(warm[:], warm[:], FT.Exp, scale=1.0)
            # persistent V-natural tiles; ones columns written once
            vns = []
            for i in range(S // 128):
                vn = vnp.tile([128, 130], bf16, tag=f"vn{i}", name=f"vn{i}")
                nc.vector.memset(vn[:, 64:65], 1.0)
                nc.vector.memset(vn[:, 129:130], 1.0)
                vns.append(vn)

            def alloc_qkv(b):
                return (
                    qk.tile([128, S], f32r, tag="qT", name=f"qT{b}"),
                    qk.tile([128, S], f32r, tag="kT", name=f"kT{b}"),
                    qk.tile([128, S], bf16, tag="vT", name=f"vT{b}"),
                )

            def qkv_dma(b, t, rep):
                tok0 = b * S + t * TB
                xth = []
                for hf in range(2):
                    x1 = xp.tile([128, KC * TB // 2], f32r, tag=f"xt{hf}",
                                 name=f"xt{rep}_{b}_{t}_{hf}")
                    nc.sync.dma_start(
                        x1[:].rearrange("p (c m) -> p c m", c=KC // 2),
                        xT.ap()[hf * (E // 2):(hf + 1) * (E // 2),
                                tok0:tok0 + TB].rearrange(
                            "(c p) m -> p c m", p=128),
                    )
                    xth.append(x1)
                return xth

            def qkv_group(b, t, tiles, xth, gi, rep):
                qT, kT, vT = tiles
                wsb, bsb, dst = (
                    (wq_sb, bq_sb, qT), (wk_sb, bk_sb, kT),
                    (wv_sb, bv_sb, vT),
                )[gi]
                ps = psA.tile([128, TB], f32, tag="mm512",
                              name=f"psqkv{rep}_{b}_{t}_{gi}")
                for kc in range(KC):
                    xsrc = xth[kc // (KC // 2)]
                    nc.tensor.matmul(
                        ps[:],
                        wsb[:, kc * 128:(kc + 1) * 128],
                        xsrc[:, (kc % (KC // 2)) * TB:
                             (kc % (KC // 2) + 1) * TB],
                        start=(kc == 0), stop=(kc == KC - 1),
                    )
                nc.vector.tensor_scalar_add(
                    dst[:, t * TB:(t + 1) * TB], ps[:], bsb[:]
                )

            def vnat(b, tiles, rep, lo=0, hi=S // 128):
                vT = tiles[2]
                for i in range(lo, hi):
                    vn = vns[i]
                    pst = psA.tile([128, 128], bf16, tag="mm512",
                                   name=f"pst{rep}_{b}_{i}")
                    nc.tensor.transpose(
                        pst[:], vT[:, i * 128:(i + 1) * 128], id_sb[:]
                    )
                    # one copy into both 64-col head groups (skips the ones
                    # columns at 64 / 129) via 3D APs
                    dst = vn[:, 0:64]
                    dst3 = bass.AP(dst.tensor, dst.offset,
                                   [dst.ap[0], [65, 2], [1, 64]])
                    src = pst[:, 0:64]
                    src3 = bass.AP(src.tensor, src.offset,
                                   [src.ap[0], [64, 2], [1, 64]])
                    nc.vector.tensor_copy(dst3, src3)

            def scores(b, qb, tiles, att, rep, fill=()):
                qT, kT, vT = tiles
                q0 = qb * QB
                nkc = (q0 + QB) // 128
                fill = list(fill)
                nf = len(fill)
                fired = 0
                pss = {}
                for kc in range(nkc):
                    kst = kc * 128
                    r0 = max(0, kst - q0)
                    for h in range(2):
                        ps_s = psS.tile([128, QB], f32, tag=f"s{h}",
                                        name=f"pss{rep}_{b}_{qb}_{kc}_{h}")
                        hs = slice(h * 64, (h + 1) * 64)
                        for (p0, p1) in splits(r0, QB):
                            nc.tensor.matmul(
                                ps_s[:, p0:p1],
                                kT[hs, kst:kst + 128],
                                qT[hs, q0 + p0:q0 + p1],
                                start=True, stop=True,
                                tile_position=(h * 64, 0),
                            )
                        pss[(kc, h)] = ps_s
                    for h in range(2):
                        ps_s = pss[(kc, h)]
                        nc.scalar.activation(
                            att[h][:, kc * QB + r0:(kc + 1) * QB],
                            ps_s[:, r0:QB],
                            FT.Exp, scale=0.125,
                        )
                        if kst >= q0:
                            blk = att[h][:, kc * QB + r0:kc * QB + r0 + 128]
                            nc.vector.tensor_tensor(
                                blk, blk, tri_sb[:],
                                op=mybir.AluOpType.mult,
                            )
                    # interleave PE fill work (next batch QKV groups) to
                    # cover the ACT exp-throughput deficit
                    want = (kc + 1) * nf // nkc
                    while fired < want:
                        fill[fired]()
                        fired += 1
                while fired < nf:
                    fill[fired]()
                    fired += 1

            def attv_qbb(b, qb, qbb, h, att, aos, rep):
                q0 = qb * QB
                qa0 = q0 + qbb * 512
                nkc_q = (qa0 + 512) // 128
                ps_o = psO.tile([65, 512], f32, tag=f"o{h}",
                                name=f"pso{rep}_{b}_{qb}_{qbb}_{h}")
                for kc in range(nkc_q):
                    kst = kc * 128
                    lo = max(qa0, kst) - qa0
                    vn = vns[kc]
                    nc.tensor.matmul(
                        ps_o[:, lo:512],
                        vn[:, h * 65:(h + 1) * 65],
                        att[h][:, kc * QB + qbb * 512 + lo:
                               kc * QB + (qbb + 1) * 512],
                        start=(kc == 0), stop=(kc == nkc_q - 1),
                    )
                rec = ms.tile([1, 512], f32, tag=f"rec{h}",
                              name=f"rec{rep}_{b}_{qb}_{qbb}_{h}")
                nc.vector.reciprocal(rec[:], ps_o[64:65, :])
                bc = ms.tile([64, 512], f32, tag=f"bc{h}",
                             name=f"bc{rep}_{b}_{qb}_{qbb}_{h}")
                nc.gpsimd.partition_broadcast(bc[:], rec[:])
                nc.vector.tensor_tensor(
                    aos[h * 64:(h + 1) * 64, qa0:qa0 + 512],
                    ps_o[0:64, :], bc[:],
                    op=mybir.AluOpType.mult,
                )

            def outproj_ec(b, half, ec, aos, rep, eng="alt"):
                t0b = b * S
                po = op.tile([128, S // 2], f32, tag="po",
                             name=f"po{rep}_{b}_{ec}_{half}")
                for tt in range(NB // 2):
                    t = half * (NB // 2) + tt
                    ps_p = psA.tile([128, TB], f32, tag="mm512",
                                    name=f"psp{rep}_{b}_{ec}_{t}")
                    nc.tensor.matmul(
                        ps_p[:],
                        wo_sb[:, ec * 128:(ec + 1) * 128],
                        aos[:, t * TB:(t + 1) * TB],
                        start=True, stop=True,
                    )
                    # copy engine: ScalarE only when not competing with
                    # the scores-loop exp FIFO
                    if eng == "alt" and (ec * 2 + tt) % 4 == 3:
                        nc.scalar.copy(
                            po[:, tt * TB:(tt + 1) * TB], ps_p[:]
                        )
                    else:
                        nc.vector.tensor_copy(
                            po[:, tt * TB:(tt + 1) * TB], ps_p[:]
                        )
                nc.sync.dma_start(
                    outp.ap()[ec * 128:(ec + 1) * 128,
                              t0b + half * (S // 2):
                              t0b + (half + 1) * (S // 2)],
                    po[:],
                )

            def outproj_half(b, half, aos, rep):
                for ec in range(KC):
                    outproj_ec(b, half, ec, aos, rep)

            for rep in range(repeats):
                # prologue: only batch-0 tokens [0,1024) serially; the rest
                # becomes fill work inside the first scores loop
                tiles = alloc_qkv(0)
                for t in (0, 1):
                    xth = xt00 if (t == 0 and rep == 0) else qkv_dma(0, t, rep)
                    for gi in range(3):
                        qkv_group(0, t, tiles, xth, gi, rep)
                vnat(0, tiles, rep, 0, 8)
                pro_fill = []
                for t in (2, 3):
                    xth = qkv_dma(0, t, rep)
                    for gi in range(3):
                        pro_fill.append(
                            (lambda t=t, xth=xth, gi=gi, tl=tiles:
                             qkv_group(0, t, tl, xth, gi, rep))
                        )
                pro_fill.append(
                    (lambda tl=tiles: vnat(0, tl, rep, 8, S // 128))
                )
                prev = None  # (b, aos) with half-1 outproj still pending
                for b in range(B):
                    nxt = b + 1 if b + 1 < B else None
                    tiles_next = alloc_qkv(nxt) if nxt is not None else None
                    aos = aop.tile([128, S], f32r, tag="ao", name=f"ao{rep}_{b}")
                    for qb in range(NQB):
                        att = [
                            atp.tile([128, 16 * QB], bf16, tag=f"att{h}",
                                     name=f"att{rep}_{b}_{qb}_{h}")
                            for h in range(2)
                        ]
                        fill = []
                        if b == 0 and qb == 0:
                            fill.extend(pro_fill)
                        if nxt is not None:
                            tls = [0] if qb == 0 else [1, 2, 3]
                            for t in tls:
                                xth = qkv_dma(nxt, t, rep)
                                for gi in range(3):
                                    fill.append(
                                        (lambda t=t, xth=xth, gi=gi:
                                         qkv_group(nxt, t, tiles_next,
                                                   xth, gi, rep))
                                    )
                        if qb == 0 and prev is not None:
                            pb, paos = prev
                            for ec in range(KC):
                                fill.append(
                                    (lambda ec=ec, pb=pb, paos=paos:
                                     outproj_ec(pb, 1, ec, paos, rep,
                                                eng="alt"))
                                )
                            prev = None
                        if qb == 1:
                            for ec in range(KC):
                                fill.append(
                                    (lambda ec=ec: outproj_ec(b, 0, ec,
                                                              aos, rep,
                                                              eng="alt"))
                                )
                        scores(b, qb, tiles, att, rep, fill)
                        for qbb in range(QB // 512):
                            for h in range(2):
                                attv_qbb(b, qb, qbb, h, att, aos, rep)
                    if nxt is not None:
                        vnat(nxt, tiles_next, rep)
                    prev = (b, aos)
                    tiles = tiles_next
                pb, paos = prev
                outproj_half(pb, 1, paos, rep)
    nc.compile()
    return nc


_CACHE = {}


def _get_nc(repeats=1):
    if repeats not in _CACHE:
        _CACHE[repeats] = build(repeats)
    return _CACHE[repeats]


def make_in_maps(x, W_qkv, b_qkv, W_out, b_out):
    x = np.asarray(x, dtype=np.float32)
    W_qkv = np.asarray(W_qkv, dtype=np.float32)
    b_qkv = np.asarray(b_qkv, dtype=np.float32)
    W_out = np.asarray(W_out, dtype=np.float32)
    xT = np.ascontiguousarray(x.reshape(TOK, E).T)
    trim = np.ascontiguousarray(
        np.triu(np.ones((128, 128), dtype=np.float32))
    ).astype(ml_dtypes.bfloat16)
    in_maps = []
    for c in range(8):
        cs = slice(c * 128, (c + 1) * 128)
        in_maps.append({
            "xT": xT,
            "wq": np.ascontiguousarray(W_qkv[:, c * 128:(c + 1) * 128]),
            "wk": np.ascontiguousarray(W_qkv[:, E + c * 128:E + (c + 1) * 128]),
            "wv": np.ascontiguousarray(
                W_qkv[:, 2 * E + c * 128:2 * E + (c + 1) * 128]),
            "wo": np.ascontiguousarray(W_out[cs, :]),
            "bq": np.ascontiguousarray(b_qkv[c * 128:(c + 1) * 128, None]),
            "bk": np.ascontiguousarray(b_qkv[E + c * 128:E + (c + 1) * 128, None]),
            "bv": np.ascontiguousarray(
                b_qkv[2 * E + c * 128:2 * E + (c + 1) * 128, None]),
            "tri": trim,
            "idd": np.eye(128, dtype=np.float32).astype(ml_dtypes.bfloat16),
        })
    return in_maps


def gather(results, b_out):
    total = np.zeros((E, TOK), dtype=np.float64)
    for c in range(8):
        total += results[c]["outp"].astype(np.float64)
    out = total.T.astype(np.float32) + np.asarray(b_out, dtype=np.float32)
    return np.ascontiguousarray(out.reshape(B, S, E)).astype(np.float32)


def kernel(x, W_qkv, b_qkv, W_out, b_out):
    nc = _get_nc(1)
    in_maps = make_in_maps(x, W_qkv, b_qkv, W_out, b_out)
    res = bass_utils.run_bass_kernel_spmd(nc, in_maps, core_ids=list(range(8)))
    return gather(res.results, b_out)


# revision 44
# speedup vs baseline: 1.6900x; 1.3577x over previous
"""Causal multi-head attention layer on 8 Trainium2 NeuronCores.

Sharding: tensor-parallel over heads (16 heads -> 2 per core).
Per core, for its 2 heads:
  qkv^T = W_slice^T @ x^T         (f32r matmuls, x pre-transposed on host)
  S^T[k,q] = K^T_chunk^T @ Q^T    (scores transposed; softmax denom via
                                   ones-column folded into V stationary)
  att^T = exp(S^T/8)  (bf16, causal-trimmed + triangular mask on diagonal)
  out^T[dv,q] = (V|1)^T-stationary @ att^T   -> row 64 = denominator
  attout^T = out^T[0:64] * bcast(1/denom)
  partial^T[e,tok] = W_out_slice chunks @ attout^T   -> DRAM
Host: sum partials over cores, transpose, + b_out.
"""
import os
import numpy as np
import ml_dtypes

import concourse.bacc as bacc
import concourse.bass as bass
import concourse.mybir as mybir
import concourse.tile as tile
from concourse import bass_utils

B, S, E, H = 4, 2048, 1024, 16
D = E // H            # 64
TOK = B * S           # 8192
KC = E // 128         # 8 emb chunks
TB = 512              # qkv token block
QB = 1024             # attention q block
NB = S // TB          # 4 token blocks per batch
NQB = S // QB         # 2 q blocks per batch

f32 = mybir.dt.float32
f32r = mybir.dt.float32r
bf16 = mybir.dt.bfloat16
FT = mybir.ActivationFunctionType


def splits(lo, hi, step=512):
    """Split [lo, hi) into pieces aligned to `step` boundaries."""
    out = []
    p = lo
    while p < hi:
        q = min((p // step + 1) * step, hi)
        out.append((p, q))
        p = q
    return out


def build(repeats: int = 1, dbg: bool = False):
    nc = bacc.Bacc("TRN2", target_bir_lowering=False, debug=False, num_devices=8)
    xT = nc.dram_tensor("xT", [E, TOK], f32r, kind="ExternalInput")
    wq = nc.dram_tensor("wq", [E, 128], f32r, kind="ExternalInput")
    wk = nc.dram_tensor("wk", [E, 128], f32r, kind="ExternalInput")
    wv = nc.dram_tensor("wv", [E, 128], f32r, kind="ExternalInput")
    wo = nc.dram_tensor("wo", [128, E], f32r, kind="ExternalInput")
    bq = nc.dram_tensor("bq", [128, 1], f32, kind="ExternalInput")
    bk = nc.dram_tensor("bk", [128, 1], f32, kind="ExternalInput")
    bv = nc.dram_tensor("bv", [128, 1], f32, kind="ExternalInput")
    tri = nc.dram_tensor("tri", [128, 128], bf16, kind="ExternalInput")
    idd = nc.dram_tensor("idd", [128, 128], bf16, kind="ExternalInput")
    outp = nc.dram_tensor("outp", [E, TOK], f32, kind="ExternalOutput")
    if dbg:
        d_q = nc.dram_tensor("d_q", [128, S], f32, kind="ExternalOutput")
        d_k = nc.dram_tensor("d_k", [128, S], f32, kind="ExternalOutput")
        d_v = nc.dram_tensor("d_v", [128, S], f32, kind="ExternalOutput")
        d_att = nc.dram_tensor("d_att", [128, 8 * QB], f32, kind="ExternalOutput")
        d_ao = nc.dram_tensor("d_ao", [128, S], f32, kind="ExternalOutput")
        d_den = nc.dram_tensor("d_den", [1, QB], f32, kind="ExternalOutput")
        d_vn = nc.dram_tensor("d_vn", [128, 130], f32, kind="ExternalOutput")

    with tile.TileContext(nc) as tc:
        with (
            tc.tile_pool(name="wp", bufs=1) as wp,
            tc.tile_pool(name="xp", bufs=2) as xp,
            tc.tile_pool(name="qk", bufs=2) as qk,
            tc.tile_pool(name="vn", bufs=1) as vnp,
            tc.tile_pool(name="at", bufs=1) as atp,
            tc.tile_pool(name="ao", bufs=2) as aop,
            tc.tile_pool(name="ms", bufs=1) as ms,
            tc.tile_pool(name="op", bufs=3) as op,
            tc.tile_pool(name="psA", bufs=2, space="PSUM") as psA,
            tc.tile_pool(name="psS", bufs=1, space="PSUM") as psS,
            tc.tile_pool(name="psO", bufs=1, space="PSUM") as psO,
        ):
            # --- constants / weights (loaded once) ---
            # first QKV token block's x slice loads FIRST so the PE can
            # start as soon as wq lands; remaining weights follow.
            xt00 = []
            for hf in range(2):
                x1 = xp.tile([128, KC * TB // 2], f32r, tag=f"xt{hf}",
                             name=f"xt_pre0_{hf}")
                nc.sync.dma_start(
                    x1[:].rearrange("p (c m) -> p c m", c=KC // 2),
                    xT.ap()[hf * (E // 2):(hf + 1) * (E // 2), 0:TB].rearrange(
                        "(c p) m -> p c m", p=128),
                )
                xt00.append(x1)
            wq_sb = wp.tile([128, E], f32r)
            wk_sb = wp.tile([128, E], f32r)
            wv_sb = wp.tile([128, E], f32r)
            wo_sb = wp.tile([128, E], f32r)
            bq_sb = wp.tile([128, 1], f32)
            bk_sb = wp.tile([128, 1], f32)
            bv_sb = wp.tile([128, 1], f32)
            for hf in range(2):
                nc.sync.dma_start(
                    wq_sb[:, hf * (E // 2):(hf + 1) * (E // 2)].rearrange(
                        "p (c m) -> p c m", c=KC // 2),
                    wq.ap()[hf * (E // 2):(hf + 1) * (E // 2), :].rearrange(
                        "(c p) m -> p c m", p=128),
                )
            nc.sync.dma_start(bq_sb[:], bq.ap())
            for wsb_, wdr_ in ((wk_sb, wk), (wv_sb, wv)):
                nc.sync.dma_start(
                    wsb_[:].rearrange("p (c m) -> p c m", c=KC),
                    wdr_.ap().rearrange("(c p) m -> p c m", p=128),
                )
            nc.sync.dma_start(wo_sb[:], wo.ap())
            nc.sync.dma_start(bk_sb[:], bk.ap())
            nc.sync.dma_start(bv_sb[:], bv.ap())
            tri_sb = wp.tile([128, 128], bf16)
            nc.sync.dma_start(tri_sb[:], tri.ap())
            id_sb = wp.tile([128, 128], bf16)
            nc.sync.dma_start(id_sb[:], idd.ap())
            # preload ACT exp table set during the prologue
            warm = wp.tile([1, 1], f32)
            nc.vector.memset(warm[:], 0.0)
            nc.scalar.activation(warm[:], warm[:], FT.Exp, scale=1.0)
            # persistent V-natural tiles; ones columns written once
            vns = []
            for i in range(S // 128):
                vn = vnp.tile([128, 130], bf16, tag=f"vn{i}", name=f"vn{i}")
                nc.vector.memset(vn[:, 64:65], 1.0)
                nc.vector.memset(vn[:, 129:130], 1.0)
                vns.append(vn)

            def alloc_qkv(b):
                return (
                    qk.tile([128, S], f32r, tag="qT", name=f"qT{b}"),
                    qk.tile([128, S], f32r, tag="kT", name=f"kT{b}"),
                    qk.tile([128, S], bf16, tag="vT", name=f"vT{b}"),
                )

            def qkv_dma(b, t, rep):
                tok0 = b * S + t * TB
                xth = []
                for hf in range(2):
                    x1 = xp.tile([128, KC * TB // 2], f32r, tag=f"xt{hf}",
                                 name=f"xt{rep}_{b}_{t}_{hf}")
                    nc.sync.dma_start(
                        x1[:].rearrange("p (c m) -> p c m", c=KC // 2),
                        xT.ap()[hf * (E // 2):(hf + 1) * (E // 2),
                                tok0:tok0 + TB].rearrange(
                            "(c p) m -> p c m", p=128),
                    )
                    xth.append(x1)
                return xth

            def qkv_group(b, t, tiles, xth, gi, rep):
                qT, kT, vT = tiles
                wsb, bsb, dst = (
                    (wq_sb, bq_sb, qT), (wk_sb, bk_sb, kT),
                    (wv_sb, bv_sb, vT),
                )[gi]
                ps = psA.tile([128, TB], f32, tag="mm512",
                              name=f"psqkv{rep}_{b}_{t}_{gi}")
                for kc in range(KC):
                    xsrc = xth[kc // (KC // 2)]
                    nc.tensor.matmul(
                        ps[:],
                        wsb[:, kc * 128:(kc + 1) * 128],
                        xsrc[:, (kc % (KC // 2)) * TB:
                             (kc % (KC // 2) + 1) * TB],
                        start=(kc == 0), stop=(kc == KC - 1),
                    )
                nc.vector.tensor_scalar_add(
                    dst[:, t * TB:(t + 1) * TB], ps[:], bsb[:]
                )

            def vnat(b, tiles, rep, lo=0, hi=S // 128):
                vT = tiles[2]
                for i in range(lo, hi):
                    vn = vns[i]
                    pst = psA.tile([128, 128], bf16, tag="mm512",
                                   name=f"pst{rep}_{b}_{i}")
                    nc.tensor.transpose(
                        pst[:], vT[:, i * 128:(i + 1) * 128], id_sb[:]
                    )
                    # one copy into both 64-col head groups (skips the ones
                    # columns at 64 / 129) via 3D APs
                    dst = vn[:, 0:64]
                    dst3 = bass.AP(dst.tensor, dst.offset,
                                   [dst.ap[0], [65, 2], [1, 64]])
                    src = pst[:, 0:64]
                    src3 = bass.AP(src.tensor, src.offset,
                                   [src.ap[0], [64, 2], [1, 64]])
                    nc.vector.tensor_copy(dst3, src3)

            def scores(b, qb, tiles, att, rep, fill=()):
                qT, kT, vT = tiles
                q0 = qb * QB
                nkc = (q0 + QB) // 128
                fill = list(fill)
                nf = len(fill)
                fired = 0
                pss = {}
                for kc in range(nkc):
                    kst = kc * 128
                    r0 = max(0, kst - q0)
                    for h in range(2):
                        ps_s = psS.tile([128, QB], f32, tag=f"s{h}",
                                        name=f"pss{rep}_{b}_{qb}_{kc}_{h}")
                        hs = slice(h * 64, (h + 1) * 64)
                        for (p0, p1) in splits(r0, QB):
                            nc.tensor.matmul(
                                ps_s[:, p0:p1],
                                kT[hs, kst:kst + 128],
                                qT[hs, q0 + p0:q0 + p1],
                                start=True, stop=True,
                                tile_position=(h * 64, 0),
                            )
                        pss[(kc, h)] = ps_s
                    for h in range(2):
                        ps_s = pss[(kc, h)]
                        nc.scalar.activation(
                            att[h][:, kc * QB + r0:(kc + 1) * QB],
                            ps_s[:, r0:QB],
                            FT.Exp, scale=0.125,
                        )
                        if kst >= q0:
                            blk = att[h][:, kc * QB + r0:kc * QB + r0 + 128]
                            nc.vector.tensor_tensor(
                                blk, blk, tri_sb[:],
                                op=mybir.AluOpType.mult,
                            )
                    # interleave PE fill work (next batch QKV groups) to
                    # cover the ACT exp-throughput deficit
                    want = (kc + 1) * nf // nkc
                    while fired < want:
                        fill[fired]()
                        fired += 1
                while fired < nf:
                    fill[fired]()
                    fired += 1

            def attv_qbb(b, qb, qbb, h, att, aos, rep):
                q0 = qb * QB
                qa0 = q0 + qbb * 512
                nkc_q = (qa0 + 512) // 128
                ps_o = psO.tile([65, 512], f32, tag=f"o{h}",
                                name=f"pso{rep}_{b}_{qb}_{qbb}_{h}")
                for kc in range(nkc_q):
                    kst = kc * 128
                    lo = max(qa0, kst) - qa0
                    vn = vns[kc]
                    nc.tensor.matmul(
                        ps_o[:, lo:512],
                        vn[:, h * 65:(h + 1) * 65],
                        att[h][:, kc * QB + qbb * 512 + lo:
                               kc * QB + (qbb + 1) * 512],
                        start=(kc == 0), stop=(kc == nkc_q - 1),
                    )
                rec = ms.tile([1, 512], f32, tag=f"rec{h}",
                              name=f"rec{rep}_{b}_{qb}_{qbb}_{h}")
                nc.vector.reciprocal(rec[:], ps_o[64:65, :])
                bc = ms.tile([64, 512], f32, tag=f"bc{h}",
                             name=f"bc{rep}_{b}_{qb}_{qbb}_{h}")
                nc.gpsimd.partition_broadcast(bc[:], rec[:])
                nc.vector.tensor_tensor(
                    aos[h * 64:(h + 1) * 64, qa0:qa0 + 512],
                    ps_o[0:64, :], bc[:],
                    op=mybir.AluOpType.mult,
                )

            def outproj_ec(b, half, ec, aos, rep, eng="alt"):
                t0b = b * S
                po = op.tile([128, S // 2], f32, tag="po",
                             name=f"po{rep}_{b}_{ec}_{half}")
                for tt in range(NB // 2):
                    t = half * (NB // 2) + tt
                    ps_p = psA.tile([128, TB], f32, tag="mm512",
                                    name=f"psp{rep}_{b}_{ec}_{t}")
                    nc.tensor.matmul(
                        ps_p[:],
                        wo_sb[:, ec * 128:(ec + 1) * 128],
                        aos[:, t * TB:(t + 1) * TB],
                        start=True, stop=True,
                    )
                    # copy engine: ScalarE only when not competing with
                    # the scores-loop exp FIFO
                    if eng == "alt" and (ec * 2 + tt) % 4 == 3:
                        nc.scalar.copy(
                            po[:, tt * TB:(tt + 1) * TB], ps_p[:]
                        )
                    else:
                        nc.vector.tensor_copy(
                            po[:, tt * TB:(tt + 1) * TB], ps_p[:]
                        )
                nc.sync.dma_start(
                    outp.ap()[ec * 128:(ec + 1) * 128,
                              t0b + half * (S // 2):
                              t0b + (half + 1) * (S // 2)],
                    po[:],
                )

            def outproj_half(b, half, aos, rep):
                for ec in range(KC):
                    outproj_ec(b, half, ec, aos, rep)

            for rep in range(repeats):
                # prologue: only batch-0 tokens [0,1024) serially; the rest
                # becomes fill work inside the first scores loop
                tiles = alloc_qkv(0)
                for t in (0, 1):
                    xth = xt00 if (t == 0 and rep == 0) else qkv_dma(0, t, rep)
                    for gi in range(3):
                        qkv_group(0, t, tiles, xth, gi, rep)
                vnat(0, tiles, rep, 0, 8)
                pro_fill = []
                for t in (2, 3):
                    xth = qkv_dma(0, t, rep)
                    for gi in range(3):
                        pro_fill.append(
                            (lambda t=t, xth=xth, gi=gi, tl=tiles:
                             qkv_group(0, t, tl, xth, gi, rep))
                        )
                pro_fill.append(
                    (lambda tl=tiles: vnat(0, tl, rep, 8, S // 128))
                )
                prev = None  # (b, aos) with half-1 outproj still pending
                for b in range(B):
                    nxt = b + 1 if b + 1 < B else None
                    tiles_next = alloc_qkv(nxt) if nxt is not None else None
                    aos = aop.tile([128, S], f32r, tag="ao", name=f"ao{rep}_{b}")
                    for qb in range(NQB):
                        att = [
                            atp.tile([128, 16 * QB], bf16, tag=f"att{h}",
                                     name=f"att{rep}_{b}_{qb}_{h}")
                            for h in range(2)
                        ]
                        fill = []
                        if b == 0 and qb == 0:
                            fill.extend(pro_fill)
                        if nxt is not None:
                            tls = [0] if qb == 0 else [1, 2, 3]
                            for t in tls:
                                xth = qkv_dma(nxt, t, rep)
                                for gi in range(3):
                                    fill.append(
                                        (lambda t=t, xth=xth, gi=gi:
                                         qkv_group(nxt, t, tiles_next,
                                                   xth, gi, rep))
                                    )
                        if qb == 0 and prev is not None:
                            pb, paos = prev
                            for ec in range(KC):
                                fill.append(
                                    (lambda ec=ec, pb=pb, paos=paos:
                                     outproj_ec(pb, 1, ec, paos, rep,
                                                eng="alt"))
                                )
                            prev = None
                        if qb == 1:
                            for ec in range(KC):
                                fill.append(
                                    (lambda ec=ec: outproj_ec(b, 0, ec,
                                                              aos, rep,
                                                              eng="alt"))
                                )
                        scores(b, qb, tiles, att, rep, fill)
                        for qbb in range(QB // 512):
                            for h in range(2):
                                attv_qbb(b, qb, qbb, h, att, aos, rep)
                    if nxt is not None:
                        vnat(nxt, tiles_next, rep)
                    prev = (b, aos)
                    tiles = tiles_next
                pb, paos = prev
                outproj_half(pb, 1, paos, rep)
    nc.compile()
    return nc


_CACHE = {}


def _get_nc(repeats=1):
    if repeats not in _CACHE:
        _CACHE[repeats] = build(repeats)
    return _CACHE[repeats]


def make_in_maps(x, W_qkv, b_qkv, W_out, b_out):
    x = np.asarray(x, dtype=np.float32)
    W_qkv = np.asarray(W_qkv, dtype=np.float32)
    b_qkv = np.asarray(b_qkv, dtype=np.float32)
    W_out = np.asarray(W_out, dtype=np.float32)
    xT = np.ascontiguousarray(x.reshape(TOK, E).T)
    trim = np.ascontiguousarray(
        np.triu(np.ones((128, 128), dtype=np.float32))
    ).astype(ml_dtypes.bfloat16)
    in_maps = []
    for c in range(8):
        cs = slice(c * 128, (c + 1) * 128)
        in_maps.append({
            "xT": xT,
            "wq": np.ascontiguousarray(W_qkv[:, c * 128:(c + 1) * 128]),
            "wk": np.ascontiguousarray(W_qkv[:, E + c * 128:E + (c + 1) * 128]),
            "wv": np.ascontiguousarray(
                W_qkv[:, 2 * E + c * 128:2 * E + (c + 1) * 128]),
            "wo": np.ascontiguousarray(W_out[cs, :]),
            "bq": np.ascontiguousarray(b_qkv[c * 128:(c + 1) * 128, None]),
            "bk": np.ascontiguousarray(b_qkv[E + c * 128:E + (c + 1) * 128, None]),
            "bv": np.ascontiguousarray(
                b_qkv[2 * E + c * 128:2 * E + (c + 1) * 128, None]),
            "tri": trim,
            "idd": np.eye(128, dtype=np.float32).astype(ml_dtypes.bfloat16),
        })
    return in_maps


def gather(results, b_out):
    total = np.zeros((E, TOK), dtype=np.float64)
    for c in range(8):
        total += results[c]["outp"].astype(np.float64)
    out = total.T.astype(np.float32) + np.asarray(b_out, dtype=np.float32)
    return np.ascontiguousarray(out.reshape(B, S, E)).astype(np.float32)


def kernel(x, W_qkv, b_qkv, W_out, b_out):
    nc = _get_nc(1)
    in_maps = make_in_maps(x, W_qkv, b_qkv, W_out, b_out)
    res = bass_utils.run_bass_kernel_spmd(nc, in_maps, core_ids=list(range(8)))
    return gather(res.results, b_out)
